# revision 13
# baseline (speedup 1.0000x reference)
"""DMP network kernel for Trainium2 (8 NeuronCores, pure data parallel).

Math: the reference is a 54->54 linear layer followed by a 301-step Euler
integration of a DMP (dynamic movement primitive). The phase variable xp and
hence the RBF activations psi are batch-independent, and the (y, z) scan is a
linear time-invariant recurrence driven by fx = (g - y0) * (w @ P_t). The
whole scan collapses to the closed form

    Y[b, d, t] = a_t * y0 + beta_t * g + (g - y0) * (w @ Q)[b, d, t]

with a, beta [T] and Q [N, T] computed on the host from c / sigma2 in float64.

Every term is linear in x except the (g - y0) * (w @ Q) product, so the device
work per 128-row batch tile is 4 fp32r matmuls from a host-transposed x tile
(bias folded in via a ones feature row):

    A [128, 304] = xT.T @ C0   -> [dcol0, dcol1, V0 (301), pad]
    B [128, 302] = xT.T @ C1   -> [V1 (301), pad]          V_d = y0 a + g beta
    C [128, 302] = xT.T @ C2   -> [G0 (301), pad]          G_d = w_d @ Q
    D [128, 302] = xT.T @ C3   -> [G1 (301), pad]

then Y_d = V_d + dcol_d * G_d via a ScalarE scaled copy (per-partition scale)
plus a VectorE add. C0/C2 live on SBUF partitions 0..54 and C1/C3 on 64..118 so
consecutive matmuls target different PE row groups and their LDWEIGHTS overlap.
"""

import os
import numpy as np

# -- problem constants (fixed by the reference) -------------------------------
N = 25
DOF = 2
TAU = 3.0
DT = 0.01
A_X = 2.0
A_Z = 48.0
B_Z = A_Z / 4.0
T = 301
D_IN = 54           # DOF * (N + 2)
B = 65536
N_CORES = 8
B_CORE = B // N_CORES          # 8192
P = 128                        # batch rows per tile
N_TILES = B_CORE // P          # 64
X_CHUNK = 8                    # tiles per input DMA
Y_CHUNK = 2                    # tiles per output DMA
D_PAD = 55                     # 54 features + ones row
T_PAD = 302                    # fp32r matmul needs an even moving-dim count
W_HI = 64                      # partition offset of the second weight copy


# -- host-side closed-form constants ------------------------------------------
def _closed_form_consts(c, sigma2):
    """a [T], beta [T], Q [N, T] in float64."""
    c = np.asarray(c, np.float64)
    sigma2 = np.asarray(sigma2, np.float64)
    alpha = DT / TAU

    xp = np.empty(T)
    xp[0] = 1.0
    for t in range(T - 1):
        xp[t + 1] = xp[t] - (A_X * xp[t] / TAU) * DT
    psi = np.exp(-0.5 * (xp[:, None] - c[None, :]) ** 2 / sigma2[None, :])  # [T, N]
    S = psi.sum(1)
    Pmat = (psi * (xp / S)[:, None]).T                                      # [N, T]

    A = np.array([[1.0, alpha], [-alpha * A_Z * B_Z, 1.0 - alpha * A_Z]])
    a = np.empty(T)
    bvec = np.empty(T)
    M = np.eye(2)
    for t in range(T):
        a[t] = M[0, 0]
        bvec[t] = M[0, 1]
        M = A @ M
    beta = A_Z * B_Z * alpha * np.concatenate([[0.0], np.cumsum(bvec)[:-1]])

    H = np.zeros((T, T))
    for t in range(1, T):
        H[:t, t] = alpha * bvec[t - 1::-1]
    Q = Pmat @ H                                                            # [N, T]
    return a, beta, Q


def _host_inputs(x, W, b, c, sigma2, scale):
    """Build per-core input maps (numpy float32)."""
    a, beta, Q = _closed_form_consts(c, sigma2)

    W2 = np.asarray(W, np.float64) * np.asarray(scale, np.float64)[:, None]
    b2 = np.asarray(b, np.float64) * np.asarray(scale, np.float64)

    # w2e[:, j] = 55-vector [W2[j, :], b2[j]] -- the ones row carries the bias
    w2e = np.concatenate([W2.T, b2[None, :]], axis=0)       # [55, 54]

    # cb [128, 2, 606]: row block 0..54 holds C0|C2, row block 64..118 C1|C3
    cb = np.zeros((P, 2, 304 + T_PAD), np.float64)
    # C0 [55, 304]: dcol0, dcol1, V0, pad
    cb[:D_PAD, 0, 0] = w2e[:, 1] - w2e[:, 0]
    cb[:D_PAD, 0, 1] = w2e[:, 28] - w2e[:, 27]
    cb[:D_PAD, 0, 2:2 + T] = np.outer(w2e[:, 0], a) + np.outer(w2e[:, 1], beta)
    # C2 [55, 302]: G0, pad
    cb[:D_PAD, 0, 304:304 + T] = w2e[:, 2:27] @ Q
    # C1 [55, 302] at partition 64..118: V1, pad
    cb[W_HI:W_HI + D_PAD, 1, 0:T] = np.outer(w2e[:, 27], a) + np.outer(w2e[:, 28], beta)
    # C3 [55, 302] at partition 64..118: G1, pad
    cb[W_HI:W_HI + D_PAD, 1, 304:304 + T] = w2e[:, 29:54] @ Q
    cb = np.ascontiguousarray(cb.reshape(P, 2 * (304 + T_PAD)).astype(np.float32))

    # host-transposed, ones-padded x: [55, B]
    xT = np.empty((D_PAD, B), np.float32)
    xT[:D_IN] = np.asarray(x, np.float32).T
    xT[D_IN] = 1.0

    in_maps = []
    for ci in range(N_CORES):
        in_maps.append({
            "x": np.ascontiguousarray(xT[:, ci * B_CORE:(ci + 1) * B_CORE]),
            "cb": cb,
        })
    return in_maps


# -- bass program --------------------------------------------------------------
_NC_CACHE = None


def _build_program():
    global _NC_CACHE
    if _NC_CACHE is not None:
        return _NC_CACHE

    import concourse.bacc as bacc
    import concourse.tile as tile
    from concourse import mybir
    from contextlib import ExitStack

    f32 = mybir.dt.float32
    f32r = mybir.dt.float32r
    CBW = 304 + T_PAD  # 606 columns per row block

    nc = bacc.Bacc(
        "TRN2",
        target_bir_lowering=False,
        debug=False,
        num_devices=N_CORES,
    )
    x_d = nc.declare_dram_parameter("x", [D_PAD, B_CORE], f32r, isOutput=False)
    cb_d = nc.declare_dram_parameter("cb", [P, 2 * CBW], f32r, isOutput=False)
    y_d = nc.declare_dram_parameter("y", [B_CORE, DOF * T], f32, isOutput=True)

    with tile.TileContext(nc) as tc, ExitStack() as ctx:
        consts = ctx.enter_context(tc.tile_pool(name="consts", bufs=1))
        xin_p = ctx.enter_context(tc.tile_pool(name="xin", bufs=3))
        tmp_p = ctx.enter_context(tc.tile_pool(name="tmp", bufs=4))
        small_p = ctx.enter_context(tc.tile_pool(name="small", bufs=4))
        yout_p = ctx.enter_context(tc.tile_pool(name="yout", bufs=3))
        ps_p = ctx.enter_context(tc.tile_pool(name="ps", bufs=8, space="PSUM"))

        cb_sb = consts.tile([P, 2, CBW], f32r)
        nc.sync.dma_start(cb_sb[:], cb_d.rearrange("p (k w) -> p k w", w=CBW))
        C0 = cb_sb[0:D_PAD, 0, 0:304]
        C2 = cb_sb[0:D_PAD, 0, 304:304 + T_PAD]
        C1 = cb_sb[W_HI:W_HI + D_PAD, 1, 0:T_PAD]
        C3 = cb_sb[W_HI:W_HI + D_PAD, 1, 304:304 + T_PAD]

        y_view = y_d.rearrange("(nt p) f -> nt p f", p=P)      # [64, 128, 602]

        ysb = None
        for ci in range(N_TILES // X_CHUNK):
            CW = X_CHUNK * P
            xin = xin_p.tile([P, CW], f32r)
            # low copy (partitions 0..54) and high copy (64..118)
            src = x_d[:, ci * CW:(ci + 1) * CW]
            # separate DMA queue from the output stream so prefetch isn't
            # stuck behind ysb completions in the Sync HWDGE FIFO
            nc.gpsimd.dma_start(xin[0:D_PAD, :], src)
            nc.gpsimd.dma_start(xin[W_HI:W_HI + D_PAD, :], src)

            for j in range(X_CHUNK):
                i = ci * X_CHUNK + j
                xa = xin[0:D_PAD, j * P:(j + 1) * P]
                xb = xin[W_HI:W_HI + D_PAD, j * P:(j + 1) * P]

                ps_a = ps_p.tile([P, 304], f32, tag="ps")
                ps_b = ps_p.tile([P, 304], f32, tag="ps")
                ps_c = ps_p.tile([P, 304], f32, tag="ps")
                ps_d = ps_p.tile([P, 304], f32, tag="ps")
                nc.tensor.matmul(ps_a[:], xa, C0, start=True, stop=True)
                nc.tensor.matmul(ps_b[:, 0:T_PAD], xb, C1, start=True, stop=True)
                nc.tensor.matmul(ps_c[:, 0:T_PAD], xa, C2, start=True, stop=True)
                nc.tensor.matmul(ps_d[:, 0:T_PAD], xb, C3, start=True, stop=True)

                dcol = small_p.tile([P, DOF], f32)
                nc.vector.tensor_copy(dcol[:], ps_a[:, 0:2])

                # t_d = G_d * dcol_d (ScalarE per-partition scaled copy)
                t0 = tmp_p.tile([P, T], f32, tag="tmp")
                t1 = tmp_p.tile([P, T], f32, tag="tmp")
                nc.scalar.mul(t0[:], ps_c[:, 0:T], dcol[:, 0:1])
                nc.scalar.mul(t1[:], ps_d[:, 0:T], dcol[:, 1:2])

                if j % Y_CHUNK == 0:
                    ysb = yout_p.tile([P, Y_CHUNK, DOF * T], f32)
                # Y_d = V_d + t_d
                nc.vector.tensor_add(ysb[:, j % Y_CHUNK, 0:T], t0[:], ps_a[:, 2:2 + T])
                nc.vector.tensor_add(ysb[:, j % Y_CHUNK, T:2 * T], t1[:], ps_b[:, 0:T])

                if j % Y_CHUNK == Y_CHUNK - 1:
                    i0 = i - (Y_CHUNK - 1)
                    dst = y_view[i0:i0 + Y_CHUNK].rearrange("n p f -> p n f")
                    nc.sync.dma_start(dst, ysb[:])

    nc.compile()
    _NC_CACHE = nc
    return nc


_LAST_RESULTS = None


def kernel(x, W, b, c, sigma2, scale):
    global _LAST_RESULTS
    from concourse.bass_utils import run_bass_kernel_spmd

    assert x.shape == (B, D_IN), x.shape
    nc = _build_program()
    in_maps = _host_inputs(x, W, b, c, sigma2, scale)
    res = run_bass_kernel_spmd(nc, in_maps, list(range(N_CORES)))
    _LAST_RESULTS = res
    out = np.concatenate([res.results[ci]["y"] for ci in range(N_CORES)], axis=0)
    return out.astype(np.float32)
